# revision 10
# baseline (speedup 1.0000x reference)
"""Trainium2 Bass kernel for nn_Blur: depthwise 4x4 separable blur.

Reference semantics:
  h: (8, 256, 64, 512) f32
  pad W circular by 1, pad H reflect by 1, depthwise conv with
  outer([1,3,3,1],[1,3,3,1])/64, VALID -> out (8, 256, 63, 511).

Strategy:
  - Batch-parallel: core i processes h[i] (256, 64, 512). The circular W
    padding is applied host-side while sharding (so each DMA is one
    contiguous [128, 514] block).
  - Per core, channels are processed two at a time: an SBUF tile
    [128 partitions = 2ch x 64 rows, 514 cols = W circularly padded].
  - The whole separable blur is 4 TensorE matmuls accumulating in PSUM:
      out = sum_dx kx[dx] * (A @ t[:, dx:dx+511])
    where A is the 126x128 block-diagonal H-conv matrix (reflect padding
    and all normalization folded in). float32r streams at 1 col/cycle.
  - ScalarE copies PSUM->SBUF; DMA writes the [126, 511] result back.
"""

import numpy as np

import concourse.bacc as bacc
import concourse.mybir as mybir
from concourse.tile import TileContext
from concourse.bass_utils import run_bass_kernel_spmd

B, C, H, W = 8, 256, 64, 512
HO, WO = H - 1, W - 1  # 63, 511
WP = W + 3  # circularly padded width (fp32r needs even N, so compute 512 cols)
N_CORES = 8
NPAIR = C // 2  # channel pairs per core


def _h_weights():
    """Stationary operand [128, 252]: cols 0:126 = A^T/64 (W-taps 1),
    cols 126:252 = 3*A^T/64 (W-taps 3), where A is the 126x128
    block-diag H-conv matrix (taps [1,3,3,1], reflect pad)."""
    k = [1.0, 3.0, 3.0, 1.0]
    A = np.zeros((HO, H), dtype=np.float64)
    for i in range(HO):
        for dy in range(4):
            j = i + dy  # index into reflect-padded H (0..65)
            m = 1 if j == 0 else (H - 2 if j == H + 1 else j - 1)
            A[i, m] += k[dy]
    A2 = np.zeros((2 * HO, 2 * H), dtype=np.float64)
    A2[:HO, :H] = A
    A2[HO:, H:] = A
    w = np.concatenate([A2.T / 64.0, 3.0 * A2.T / 64.0], axis=1)
    return np.ascontiguousarray(w, dtype=np.float32)


def _build_nc():
    nc = bacc.Bacc()
    h = nc.declare_dram_parameter("h", [C * H, WP], mybir.dt.float32r, isOutput=False)
    w = nc.declare_dram_parameter("w", [128, 4 * HO], mybir.dt.float32r, isOutput=False)
    out = nc.declare_dram_parameter("out", [C * HO, WO], mybir.dt.float32, isOutput=True)

    with TileContext(nc) as tc:
        with (
            tc.tile_pool(name="wpool", bufs=1) as wpool,
            tc.tile_pool(name="inpool", bufs=6) as inpool,
            tc.tile_pool(name="psum", bufs=6, space="PSUM") as psum,
            tc.tile_pool(name="outpool", bufs=6) as outpool,
        ):
            w_t = wpool.tile([128, 4 * HO], mybir.dt.float32r, name="w_t")
            nc.sync.dma_start(out=w_t[:], in_=w[:])
            wa = w_t[:, 0 : 2 * HO]
            wb = w_t[:, 2 * HO : 4 * HO]
            for cp in range(NPAIR):
                t = inpool.tile([128, WP], mybir.dt.float32r, name="t", tag="t")
                nc.sync.dma_start(out=t[:], in_=h[128 * cp : 128 * (cp + 1), :])
                # fp32r matmuls need an even moving-column count: compute 512
                # output cols and discard the last one at the PSUM->SBUF copy.
                p = psum.tile([2 * HO, W], mybir.dt.float32, name="p", tag="p")
                nc.tensor.matmul(p[:], lhsT=wa, rhs=t[:, 0:W], start=True, stop=False)
                nc.tensor.matmul(p[:], lhsT=wb, rhs=t[:, 1 : 1 + W], start=False, stop=False)
                nc.tensor.matmul(p[:], lhsT=wb, rhs=t[:, 2 : 2 + W], start=False, stop=False)
                nc.tensor.matmul(p[:], lhsT=wa, rhs=t[:, 3 : 3 + W], start=False, stop=True)
                o = outpool.tile([2 * HO, WO], mybir.dt.float32, name="o", tag="o")
                nc.scalar.copy(o[:], p[:, 0:WO])
                nc.sync.dma_start(out=out[2 * HO * cp : 2 * HO * (cp + 1), :], in_=o[:])
    if not nc.is_finalized():
        nc.finalize()  # Bacc.finalize runs the wait-splitting + reg-alloc passes
    return nc


_NC_CACHE = None


def _get_nc():
    global _NC_CACHE
    if _NC_CACHE is None:
        _NC_CACHE = _build_nc()
    return _NC_CACHE


def _shard_inputs(h):
    """Per-core input maps: batch elem i -> core i, W circularly padded."""
    h = np.asarray(h, dtype=np.float32)
    hp = np.empty((B, C, H, WP), dtype=np.float32)
    hp[..., 1 : W + 1] = h
    hp[..., 0] = h[..., W - 1]
    hp[..., W + 1] = h[..., 0]
    hp[..., W + 2] = h[..., 1]
    w = _h_weights()
    return [{"h": hp[i].reshape(C * H, WP), "w": w} for i in range(N_CORES)]


def kernel(h, _trace=False):
    assert h.shape == (B, C, H, W)
    in_maps = _shard_inputs(h)
    nc = _get_nc()
    res = run_bass_kernel_spmd(nc, in_maps, list(range(N_CORES)), trace=_trace)
    out = np.stack(
        [res.results[i]["out"].reshape(C, HO, WO) for i in range(N_CORES)], axis=0
    )
    if _trace:
        return out, res
    return out


# revision 12
# speedup vs baseline: 144.4868x; 144.4868x over previous
"""Trainium2 Bass kernel for nn_Blur: depthwise 4x4 separable blur.

Reference semantics:
  h: (8, 256, 64, 512) f32
  pad W circular by 1, pad H reflect by 1, depthwise conv with
  outer([1,3,3,1],[1,3,3,1])/64, VALID -> out (8, 256, 63, 511).

Strategy:
  - Batch-parallel: core i processes h[i] (256, 64, 512). The circular W
    padding is applied host-side while sharding (so each DMA is one
    contiguous [128, 514] block).
  - Per core, channels are processed two at a time: an SBUF tile
    [128 partitions = 2ch x 64 rows, 514 cols = W circularly padded].
  - The whole separable blur is 4 TensorE matmuls accumulating in PSUM:
      out = sum_dx kx[dx] * (A @ t[:, dx:dx+511])
    where A is the 126x128 block-diagonal H-conv matrix (reflect padding
    and all normalization folded in). float32r streams at 1 col/cycle.
  - ScalarE copies PSUM->SBUF; DMA writes the [126, 511] result back.
"""

import numpy as np

import concourse.bacc as bacc
import concourse.mybir as mybir
from concourse.tile import TileContext
from concourse.bass_utils import run_bass_kernel_spmd

B, C, H, W = 8, 256, 64, 512
HO, WO = H - 1, W - 1  # 63, 511
WP = W + 3  # circularly padded width (fp32r needs even N, so compute 512 cols)
N_CORES = 8
NPAIR = C // 2  # channel pairs per core


def _h_weights():
    """Stationary operand [128, 252]: cols 0:126 = A^T/64 (W-taps 1),
    cols 126:252 = 3*A^T/64 (W-taps 3), where A is the 126x128
    block-diag H-conv matrix (taps [1,3,3,1], reflect pad)."""
    k = [1.0, 3.0, 3.0, 1.0]
    A = np.zeros((HO, H), dtype=np.float64)
    for i in range(HO):
        for dy in range(4):
            j = i + dy  # index into reflect-padded H (0..65)
            m = 1 if j == 0 else (H - 2 if j == H + 1 else j - 1)
            A[i, m] += k[dy]
    A2 = np.zeros((2 * HO, 2 * H), dtype=np.float64)
    A2[:HO, :H] = A
    A2[HO:, H:] = A
    w = np.concatenate([A2.T / 64.0, 3.0 * A2.T / 64.0], axis=1)
    return np.ascontiguousarray(w, dtype=np.float32)


def _build_nc(repeat=1):
    """repeat>1 re-runs the whole per-core workload (writing the same
    outputs) inside one NEFF — used only for slope-based timing."""
    nc = bacc.Bacc()
    h = nc.declare_dram_parameter("h", [C * H, WP], mybir.dt.float32r, isOutput=False)
    w = nc.declare_dram_parameter("w", [128, 4 * HO], mybir.dt.float32r, isOutput=False)
    out = nc.declare_dram_parameter("out", [C * HO, WO], mybir.dt.float32, isOutput=True)

    with TileContext(nc) as tc:
        with (
            tc.tile_pool(name="wpool", bufs=1) as wpool,
            tc.tile_pool(name="inpool", bufs=6) as inpool,
            tc.tile_pool(name="psum", bufs=6, space="PSUM") as psum,
            tc.tile_pool(name="outpool", bufs=6) as outpool,
        ):
            w_t = wpool.tile([128, 4 * HO], mybir.dt.float32r, name="w_t")
            nc.sync.dma_start(out=w_t[:], in_=w[:])
            wa = w_t[:, 0 : 2 * HO]
            wb = w_t[:, 2 * HO : 4 * HO]
            for cp in [cp for _ in range(repeat) for cp in range(NPAIR)]:
                t = inpool.tile([128, WP], mybir.dt.float32r, name="t", tag="t")
                nc.sync.dma_start(out=t[:], in_=h[128 * cp : 128 * (cp + 1), :])
                # fp32r matmuls need an even moving-column count: compute 512
                # output cols and discard the last one at the PSUM->SBUF copy.
                p = psum.tile([2 * HO, W], mybir.dt.float32, name="p", tag="p")
                nc.tensor.matmul(p[:], lhsT=wa, rhs=t[:, 0:W], start=True, stop=False)
                nc.tensor.matmul(p[:], lhsT=wb, rhs=t[:, 1 : 1 + W], start=False, stop=False)
                nc.tensor.matmul(p[:], lhsT=wb, rhs=t[:, 2 : 2 + W], start=False, stop=False)
                nc.tensor.matmul(p[:], lhsT=wa, rhs=t[:, 3 : 3 + W], start=False, stop=True)
                o = outpool.tile([2 * HO, WO], mybir.dt.float32, name="o", tag="o")
                nc.scalar.copy(o[:], p[:, 0:WO])
                nc.sync.dma_start(out=out[2 * HO * cp : 2 * HO * (cp + 1), :], in_=o[:])
    if not nc.is_finalized():
        nc.finalize()  # Bacc.finalize runs the wait-splitting + reg-alloc passes
    return nc


_NC_CACHE = None


def _get_nc():
    global _NC_CACHE
    if _NC_CACHE is None:
        _NC_CACHE = _build_nc()
    return _NC_CACHE


def _shard_inputs(h):
    """Per-core input maps: batch elem i -> core i, W circularly padded."""
    h = np.asarray(h, dtype=np.float32)
    hp = np.empty((B, C, H, WP), dtype=np.float32)
    hp[..., 1 : W + 1] = h
    hp[..., 0] = h[..., W - 1]
    hp[..., W + 1] = h[..., 0]
    hp[..., W + 2] = h[..., 1]
    w = _h_weights()
    return [{"h": hp[i].reshape(C * H, WP), "w": w} for i in range(N_CORES)]


def kernel(h, _trace=False):
    assert h.shape == (B, C, H, W)
    in_maps = _shard_inputs(h)
    nc = _get_nc()
    res = run_bass_kernel_spmd(nc, in_maps, list(range(N_CORES)), trace=_trace)
    out = np.stack(
        [res.results[i]["out"].reshape(C, HO, WO) for i in range(N_CORES)], axis=0
    )
    if _trace:
        return out, res
    return out


# revision 15
# speedup vs baseline: 225.8202x; 1.5629x over previous
"""Trainium2 Bass kernel for nn_Blur: depthwise 4x4 separable blur.

Reference semantics:
  h: (8, 256, 64, 512) f32
  pad W circular by 1, pad H reflect by 1, depthwise conv with
  outer([1,3,3,1],[1,3,3,1])/64, VALID -> out (8, 256, 63, 511).

Strategy (DMA-bound; ~66 MB/core must move once):
  - Batch-parallel: core i processes h[i] (256, 64, 512).
  - Host-side shard prep pads W circularly into 528-col rows (64B-aligned
    row starts; aligned 2112B-row reads nearly double HBM efficiency vs
    odd-size rows).
  - Per core, channels are processed two at a time: an SBUF tile
    [128 partitions = 2ch x 64 rows, 528 cols].
  - The whole separable blur is 4 TensorE matmuls accumulating in PSUM:
      out = sum_dx kx[dx] * (A @ t[:, dx:dx+512])
    where A is the 126x128 block-diagonal H-conv matrix (reflect padding
    and all normalization folded in). float32r (TF32) streams at
    1 col/cycle; even moving-column count required, so 512 output cols
    are computed and the 512th is stripped on the host.
  - ScalarE copies PSUM->SBUF. Input DMAs ride the SP HWDGE ring and
    output DMAs the ACT ring, so the two directions issue in parallel.
  - Output DRAM rows are 512 cols (2048B, burst-aligned); host strips to
    511.
"""

import numpy as np

import concourse.bacc as bacc
import concourse.mybir as mybir
from concourse.tile import TileContext
from concourse.bass_utils import run_bass_kernel_spmd

B, C, H, W = 8, 256, 64, 512
HO, WO = H - 1, W - 1  # 63, 511
WPIN = 528  # circularly padded + 64B-aligned input row width (>= 515)
N_CORES = 8
NPAIR = C // 2  # channel pairs per core


def _h_weights():
    """Stationary operand [128, 252]: cols 0:126 = A^T/64 (W-taps 1),
    cols 126:252 = 3*A^T/64 (W-taps 3), where A is the 126x128
    block-diag H-conv matrix (taps [1,3,3,1], reflect pad)."""
    k = [1.0, 3.0, 3.0, 1.0]
    A = np.zeros((HO, H), dtype=np.float64)
    for i in range(HO):
        for dy in range(4):
            j = i + dy  # index into reflect-padded H (0..65)
            m = 1 if j == 0 else (H - 2 if j == H + 1 else j - 1)
            A[i, m] += k[dy]
    A2 = np.zeros((2 * HO, 2 * H), dtype=np.float64)
    A2[:HO, :H] = A
    A2[HO:, H:] = A
    w = np.concatenate([A2.T / 64.0, 3.0 * A2.T / 64.0], axis=1)
    return np.ascontiguousarray(w, dtype=np.float32)


def _build_nc(repeat=1):
    """repeat>1 re-runs the per-core workload inside a hardware loop
    (writing the same outputs) — used only for slope-based timing."""
    nc = bacc.Bacc()
    h = nc.declare_dram_parameter("h", [C * H, WPIN], mybir.dt.float32r, isOutput=False)
    w = nc.declare_dram_parameter("w", [128, 4 * HO], mybir.dt.float32r, isOutput=False)
    out = nc.declare_dram_parameter("out", [C * HO, W], mybir.dt.float32, isOutput=True)

    with TileContext(nc) as tc:
        with (
            tc.tile_pool(name="wpool", bufs=1) as wpool,
            tc.tile_pool(name="inpool", bufs=6) as inpool,
            tc.tile_pool(name="psum", bufs=6, space="PSUM") as psum,
            tc.tile_pool(name="outpool", bufs=6) as outpool,
        ):
            w_t = wpool.tile([128, 4 * HO], mybir.dt.float32r, name="w_t")
            nc.sync.dma_start(out=w_t[:], in_=w[:])
            wa = w_t[:, 0 : 2 * HO]
            wb = w_t[:, 2 * HO : 4 * HO]

            def one_pass():
                for cp in range(NPAIR):
                    t = inpool.tile([128, WPIN], mybir.dt.float32r, name="t", tag="t")
                    nc.sync.dma_start(out=t[:], in_=h[128 * cp : 128 * (cp + 1), :])
                    p = psum.tile([2 * HO, W], mybir.dt.float32, name="p", tag="p")
                    nc.tensor.matmul(p[:], lhsT=wa, rhs=t[:, 0:W], start=True, stop=False)
                    nc.tensor.matmul(p[:], lhsT=wb, rhs=t[:, 1 : 1 + W], start=False, stop=False)
                    nc.tensor.matmul(p[:], lhsT=wb, rhs=t[:, 2 : 2 + W], start=False, stop=False)
                    nc.tensor.matmul(p[:], lhsT=wa, rhs=t[:, 3 : 3 + W], start=False, stop=True)
                    o = outpool.tile([2 * HO, W], mybir.dt.float32, name="o", tag="o")
                    nc.scalar.copy(o[:], p[:])
                    nc.scalar.dma_start(
                        out=out[2 * HO * cp : 2 * HO * (cp + 1), :], in_=o[:]
                    )

            if repeat > 1:
                with tc.For_i(0, repeat, 1):
                    one_pass()
            else:
                one_pass()
    if not nc.is_finalized():
        nc.finalize()  # Bacc.finalize runs the wait-splitting + reg-alloc passes
    return nc


_NC_CACHE = None


def _get_nc():
    global _NC_CACHE
    if _NC_CACHE is None:
        _NC_CACHE = _build_nc()
    return _NC_CACHE


def _shard_inputs(h):
    """Per-core input maps: batch elem i -> core i, W circularly padded
    into 64B-aligned rows (cols 515..527 left uninitialized, never read)."""
    h = np.asarray(h, dtype=np.float32)
    hp = np.empty((B, C, H, WPIN), dtype=np.float32)
    hp[..., 1 : W + 1] = h
    hp[..., 0] = h[..., W - 1]
    hp[..., W + 1] = h[..., 0]
    hp[..., W + 2] = h[..., 1]
    hp[..., W + 3 :] = 0.0
    w = _h_weights()
    return [{"h": hp[i].reshape(C * H, WPIN), "w": w} for i in range(N_CORES)]


def kernel(h, _trace=False):
    assert h.shape == (B, C, H, W)
    in_maps = _shard_inputs(h)
    nc = _get_nc()
    res = run_bass_kernel_spmd(nc, in_maps, list(range(N_CORES)), trace=_trace)
    out = np.stack(
        [res.results[i]["out"].reshape(C, HO, W)[:, :, :WO] for i in range(N_CORES)],
        axis=0,
    )
    out = np.ascontiguousarray(out)
    if _trace:
        return out, res
    return out


# revision 16
# speedup vs baseline: 329.1011x; 1.4574x over previous
"""Trainium2 Bass kernel for nn_Blur: depthwise 4x4 separable blur.

Reference semantics:
  h: (8, 256, 64, 512) f32
  pad W circular by 1, pad H reflect by 1, depthwise conv with
  outer([1,3,3,1],[1,3,3,1])/64, VALID -> out (8, 256, 63, 511).

Strategy (DMA-bound; ~66 MB/core must move once):
  - Batch-parallel: core i processes h[i] (256, 64, 512).
  - Input is read in its natural layout ([128, 512] blocks, 2048B
    aligned rows; aligned rows nearly double HBM efficiency vs odd-size
    rows). VectorE fills the three circular-wrap columns on chip.
  - Per core, channels are processed two at a time: an SBUF tile
    [128 partitions = 2ch x 64 rows, 515 cols = wrap + 512 + 2 wrap].
  - The whole separable blur is 4 TensorE matmuls accumulating in PSUM:
      out = sum_dx kx[dx] * (A @ t[:, dx:dx+512])
    where A is the 126x128 block-diagonal H-conv matrix (reflect padding
    and all normalization folded in). float32r (TF32) streams at
    1 col/cycle; even moving-column count required, so 512 output cols
    are computed and the 512th is stripped on the host.
  - ScalarE copies PSUM->SBUF. Input DMAs ride the SP HWDGE ring and
    output DMAs the ACT ring, so the two directions issue in parallel.
  - Output DRAM rows are 512 cols (2048B, burst-aligned); host strips to
    511.
"""

import numpy as np

import concourse.bacc as bacc
import concourse.mybir as mybir
from concourse.tile import TileContext
from concourse.bass_utils import run_bass_kernel_spmd

B, C, H, W = 8, 256, 64, 512
HO, WO = H - 1, W - 1  # 63, 511
WPIN = 515  # on-chip padded row width: [c511 | c0..c511 | c0 c1]
N_CORES = 8
NPAIR = C // 2  # channel pairs per core


def _h_weights():
    """Stationary operand [128, 252]: cols 0:126 = A^T/64 (W-taps 1),
    cols 126:252 = 3*A^T/64 (W-taps 3), where A is the 126x128
    block-diag H-conv matrix (taps [1,3,3,1], reflect pad)."""
    k = [1.0, 3.0, 3.0, 1.0]
    A = np.zeros((HO, H), dtype=np.float64)
    for i in range(HO):
        for dy in range(4):
            j = i + dy  # index into reflect-padded H (0..65)
            m = 1 if j == 0 else (H - 2 if j == H + 1 else j - 1)
            A[i, m] += k[dy]
    A2 = np.zeros((2 * HO, 2 * H), dtype=np.float64)
    A2[:HO, :H] = A
    A2[HO:, H:] = A
    w = np.concatenate([A2.T / 64.0, 3.0 * A2.T / 64.0], axis=1)
    return np.ascontiguousarray(w, dtype=np.float32)


def _build_nc(repeat=1):
    """repeat>1 re-runs the per-core workload inside a hardware loop
    (writing the same outputs) — used only for slope-based timing."""
    nc = bacc.Bacc()
    h = nc.declare_dram_parameter("h", [C * H, W], mybir.dt.float32r, isOutput=False)
    w = nc.declare_dram_parameter("w", [128, 4 * HO], mybir.dt.float32r, isOutput=False)
    out = nc.declare_dram_parameter("out", [C * HO, W], mybir.dt.float32, isOutput=True)

    with TileContext(nc) as tc:
        with (
            tc.tile_pool(name="wpool", bufs=1) as wpool,
            tc.tile_pool(name="inpool", bufs=6) as inpool,
            tc.tile_pool(name="psum", bufs=6, space="PSUM") as psum,
            tc.tile_pool(name="outpool", bufs=6) as outpool,
        ):
            w_t = wpool.tile([128, 4 * HO], mybir.dt.float32r, name="w_t")
            nc.sync.dma_start(out=w_t[:], in_=w[:])
            wa = w_t[:, 0 : 2 * HO]
            wb = w_t[:, 2 * HO : 4 * HO]

            def one_pass():
                for cp in range(NPAIR):
                    t = inpool.tile([128, WPIN], mybir.dt.float32r, name="t", tag="t")
                    nc.sync.dma_start(out=t[:, 1:513], in_=h[128 * cp : 128 * (cp + 1), :])
                    nc.vector.tensor_copy(t[:, 0:1], t[:, 512:513])
                    nc.vector.tensor_copy(t[:, 513:514], t[:, 1:2])
                    nc.vector.tensor_copy(t[:, 514:515], t[:, 2:3])
                    p = psum.tile([2 * HO, W], mybir.dt.float32, name="p", tag="p")
                    nc.tensor.matmul(p[:], lhsT=wa, rhs=t[:, 0:W], start=True, stop=False)
                    nc.tensor.matmul(p[:], lhsT=wb, rhs=t[:, 1 : 1 + W], start=False, stop=False)
                    nc.tensor.matmul(p[:], lhsT=wb, rhs=t[:, 2 : 2 + W], start=False, stop=False)
                    nc.tensor.matmul(p[:], lhsT=wa, rhs=t[:, 3 : 3 + W], start=False, stop=True)
                    o = outpool.tile([2 * HO, W], mybir.dt.float32, name="o", tag="o")
                    nc.scalar.copy(o[:], p[:])
                    nc.scalar.dma_start(
                        out=out[2 * HO * cp : 2 * HO * (cp + 1), :], in_=o[:]
                    )

            if repeat > 1:
                with tc.For_i(0, repeat, 1):
                    one_pass()
            else:
                one_pass()
    if not nc.is_finalized():
        nc.finalize()  # Bacc.finalize runs the wait-splitting + reg-alloc passes
    return nc


_NC_CACHE = None


def _get_nc():
    global _NC_CACHE
    if _NC_CACHE is None:
        _NC_CACHE = _build_nc()
    return _NC_CACHE


def _shard_inputs(h):
    """Per-core input maps: batch elem i -> core i, natural layout
    (circular wrap columns are filled on chip)."""
    h = np.ascontiguousarray(h, dtype=np.float32)
    w = _h_weights()
    return [{"h": h[i].reshape(C * H, W), "w": w} for i in range(N_CORES)]


def kernel(h, _trace=False):
    assert h.shape == (B, C, H, W)
    in_maps = _shard_inputs(h)
    nc = _get_nc()
    res = run_bass_kernel_spmd(nc, in_maps, list(range(N_CORES)), trace=_trace)
    out = np.stack(
        [res.results[i]["out"].reshape(C, HO, W)[:, :, :WO] for i in range(N_CORES)],
        axis=0,
    )
    out = np.ascontiguousarray(out)
    if _trace:
        return out, res
    return out
